# revision 1
# baseline (speedup 1.0000x reference)
"""Sharded masked dot-product attention for 8 TRN2 NeuronCores.

Problem: B=64, Lq=Lk=1024, D=64 fp32 attention with per-batch valid_lens
masking (scores at k >= valid_len forced to -1e6 before softmax).

Strategy
--------
Batch dim is sharded 8 ways (8 batches per core, one per "slot").  On the
host we:
  * compute nkb[b] = ceil(valid_len[b]/128) -- number of 128-wide k-blocks
    that can contribute anything to the output,
  * sort batches by nkb descending and deal them round-robin to
    (slot j, core c) = order[j*8 + c], so the compile-time per-slot block
    count nkb_slot[j] = max_c nkb (= nkb[order[8j]], sorted) is tight,
  * pre-transpose Q and K to [d, L] layout (the PE contracts over the
    partition dim, so both matmul operands need d on partitions), packing
    two slots per 128-partition DRAM plane for full-rate DMA,
  * append a ones-column to V (V_aug[k, 65]); the AV matmul then yields
    the softmax denominator in output row 64 for free,
  * build per-k additive mask bias (0 or -1e6) folded into the EXP
    activation's bias operand (also for free).

Device, per slot:
  S^T[k, q]  = K^T_blk.T @ Q^T           (PSUM, k on partitions)
  A^T[k, q]  = exp(0.125*S^T + bias[k])  (ScalarE, mask+scale folded in)
  O'[65, q] += V_aug_blk.T @ A^T_blk     (PSUM accumulate over k-blocks)
  out^T      = O'[0:64] * (1 / O'[64])   (DVE recip + GpSimd partition
                                          broadcast + DVE multiply)
Host unshards and transposes out^T -> [B, Lq, D].
"""

import numpy as np

import concourse.mybir as mybir
import concourse.tile as tile
from concourse import bacc
from concourse.bass_utils import run_bass_kernel_spmd

B, LQ, LK, D = 64, 1024, 1024, 64
NCORES = 8
SLOTS = 8                 # batches per core
KB = 128                  # k-block size (partition dim of S^T)
NKB_MAX = LK // KB        # 8
QH = 512                  # q processed per matmul (fp32 max moving free dim)
MASK_VALUE = -1000000.0
SCALE = 1.0 / np.sqrt(D)  # 0.125, folded into the exp activation

F32 = mybir.dt.float32

# Matmul-operand dtype: float32 (exact, 1/4-rate PE), float32r (full-rate
# fp32-storage mode), or bfloat16 (full-rate, half DMA traffic).
MM_DT = mybir.dt.float32r


def _mm_np_dtype():
    import ml_dtypes

    return ml_dtypes.bfloat16 if MM_DT == mybir.dt.bfloat16 else np.float32


def _emit(ctx, tc, aps, nkb_slot, rep=0):
    nc = tc.nc
    qt_d, kt_d, va_d, mb_d, ot_d = aps

    io = ctx.enter_context(tc.tile_pool(name=f"io{rep}", bufs=2))
    apool = ctx.enter_context(tc.tile_pool(name=f"apool{rep}", bufs=2))
    psum = ctx.enter_context(tc.tile_pool(name=f"psum{rep}", bufs=2, space="PSUM"))

    # Warm-up activation: forces the Exp table load at t=0, overlapping the
    # initial input DMAs instead of stalling the first real exp.
    warm = io.tile([1, 1], F32, tag="warm", bufs=1)
    nc.vector.memset(warm, 0.0)
    nc.scalar.activation(
        out=warm, in_=warm, func=mybir.ActivationFunctionType.Exp
    )

    # All 8 slots' mask-bias columns in one tiny early DMA (keeps the first
    # exp's bias off the tail of the bulk-load FIFO).
    mb_all = io.tile([128, SLOTS * NKB_MAX], F32, tag="mb", bufs=1)
    nc.sync.dma_start(out=mb_all, in_=mb_d)

    # kt block 0 / qt q-half 0 live in their own tiles so the first QK
    # matmul's dependency covers only the small leading DMAs (Tile tracks
    # dependencies per tile, not per byte range).
    qt_t = {}   # p -> [qh0_tile, qh1_tile]
    kt0_t = {}  # p -> [128, KB] first k-block
    ktr_t = {}  # p -> [128, kcols-KB] remaining k-blocks
    for p in range(SLOTS // 2):
        # Pair-packed [128, L] planes: partitions 0:64 = slot 2p,
        # 64:128 = slot 2p+1.
        kt0_t[p] = io.tile([128, KB], MM_DT, tag="kt0", bufs=2,
                           name=f"kt0_sb{p}")
        nc.sync.dma_start(out=kt0_t[p], in_=kt_d[p][:, :KB])
        qt_t[p] = []
        for qh in range(LQ // QH):
            t = io.tile([128, QH], MM_DT, tag=f"qt{qh}", bufs=2,
                        name=f"qt_sb{p}_{qh}")
            nc.sync.dma_start(out=t, in_=qt_d[p][:, qh * QH:(qh + 1) * QH])
            qt_t[p].append(t)
        kcols = nkb_slot[2 * p] * KB
        if kcols > KB:
            ktr_t[p] = io.tile([128, kcols - KB], MM_DT, tag="ktr",
                               bufs=2, name=f"ktr_sb{p}")
            nc.sync.dma_start(out=ktr_t[p], in_=kt_d[p][:, KB:kcols])

        # Per slot: QK matmuls -> exp -> AV accumulation per k-block (keeps
        # the trailing dependence chain after the last exp short), then
        # normalize + store.  Slot 2p uses PE row-group 0, slot 2p+1
        # row-group 64 (tile_position auto-derived from base_partition).
        for half in range(2):
            j = 2 * p + half
            nkb = nkb_slot[j]
            base = 64 * half
            vat = io.tile([128, nkb, D + 1], MM_DT, tag="va", bufs=2,
                          name=f"va_sb{j}")
            nc.sync.dma_start(
                out=vat, in_=va_d[j, :nkb].rearrange("n p d -> p n d")
            )
            avs = [psum.tile([D + 1, QH], F32, tag="av", bufs=4,
                             name=f"av{j}_{qh}") for qh in range(LQ // QH)]
            for kb in range(nkb):
                kt_ap = (kt0_t[p] if kb == 0
                         else ktr_t[p][:, (kb - 1) * KB:kb * KB])
                bias_ap = mb_all[:, j * NKB_MAX + kb:j * NKB_MAX + kb + 1]
                if rep == 0 and j == 0 and kb == 0:
                    # Very first block: per-q-half S/A tiles so the first
                    # exp only depends on the qh0 loads (deps are
                    # tile-granular) -- starts the ACT stream ~1.5us sooner.
                    for qh in range(LQ // QH):
                        st_h = psum.tile([128, QH], F32, tag="st", bufs=2,
                                         name=f"st0h{qh}")
                        nc.tensor.matmul(
                            st_h,
                            lhsT=kt_ap[base:base + 64, :],
                            rhs=qt_t[p][qh][base:base + 64, :],
                            start=True,
                            stop=True,
                        )
                        at_h = apool.tile([128, QH], MM_DT, tag="at", bufs=8,
                                          name=f"at0h{qh}")
                        nc.scalar.activation(
                            out=at_h,
                            in_=st_h,
                            func=mybir.ActivationFunctionType.Exp,
                            bias=bias_ap,
                            scale=SCALE,
                        )
                        nc.tensor.matmul(
                            avs[qh],
                            lhsT=vat[:, kb, :],
                            rhs=at_h,
                            start=True,
                            stop=(nkb == 1),
                        )
                    continue
                st = psum.tile([128, LQ], F32, tag="st", bufs=2,
                               name=f"st{j}_{kb}")
                for qh in range(LQ // QH):
                    nc.tensor.matmul(
                        st[:, qh * QH:(qh + 1) * QH],
                        lhsT=kt_ap[base:base + 64, :],
                        rhs=qt_t[p][qh][base:base + 64, :],
                        start=True,
                        stop=True,
                    )
                at = apool.tile([128, LQ], MM_DT, tag="at", bufs=8,
                                name=f"at{j}_{kb}")
                nc.scalar.activation(
                    out=at,
                    in_=st,
                    func=mybir.ActivationFunctionType.Exp,
                    bias=bias_ap,
                    scale=SCALE,
                )
                for qh in range(LQ // QH):
                    nc.tensor.matmul(
                        avs[qh],
                        lhsT=vat[:, kb, :],
                        rhs=at[:, qh * QH:(qh + 1) * QH],
                        start=(kb == 0),
                        stop=(kb == nkb - 1),
                    )

            for qh in range(LQ // QH):
                av = avs[qh]
                rec = io.tile([1, QH], F32, tag="rec", bufs=2,
                              name=f"rec{j}_{qh}")
                nc.vector.reciprocal(out=rec, in_=av[D:D + 1, :])
                rb = io.tile([64, QH], F32, tag="rb", bufs=2,
                             name=f"rb{j}_{qh}")
                nc.gpsimd.partition_broadcast(rb, rec)
                ot_t = io.tile([64, QH], F32, tag="ot", bufs=4,
                               name=f"ot{j}_{qh}")
                nc.vector.tensor_mul(ot_t, av[0:D, :], rb)
                nc.sync.dma_start(out=ot_d[j, :, qh * QH:(qh + 1) * QH],
                                  in_=ot_t)


def build_program(nkb_slot, repeat=1):
    """Build + compile the per-core Bass program for the given per-slot
    k-block counts (identical across cores -- SPMD).  repeat>1 re-emits the
    whole body (benchmarking only: slope between repeat counts cancels the
    constant NEFF launch overhead)."""
    from contextlib import ExitStack

    nc = bacc.Bacc(
        "TRN2", target_bir_lowering=False, debug=False, num_devices=NCORES
    )
    qt = nc.dram_tensor("qt", [4, 128, LQ], MM_DT, kind="ExternalInput").ap()
    kt = nc.dram_tensor("kt", [4, 128, LK], MM_DT, kind="ExternalInput").ap()
    va = nc.dram_tensor(
        "va", [SLOTS, NKB_MAX, KB, D + 1], MM_DT, kind="ExternalInput"
    ).ap()
    mb = nc.dram_tensor("mb", [KB, SLOTS * NKB_MAX], F32, kind="ExternalInput").ap()
    ot = nc.dram_tensor("ot", [SLOTS, D, LQ], F32, kind="ExternalOutput").ap()

    with tile.TileContext(nc) as tc:
        for r in range(repeat):
            with ExitStack() as ctx:
                _emit(ctx, tc, (qt, kt, va, mb, ot), nkb_slot, rep=r)
    nc.compile()
    return nc


def shard_inputs(queries, keys, values, valid_lens):
    """Returns (nkb_slot tuple, in_maps list, assignment array).

    assignment[c, j] = original batch index handled by core c, slot j."""
    queries = np.asarray(queries, dtype=np.float32)
    keys = np.asarray(keys, dtype=np.float32)
    values = np.asarray(values, dtype=np.float32)
    vl = np.asarray(valid_lens).astype(np.int64).reshape(B)
    vl = np.clip(vl, 1, LK)

    nkb = np.clip((vl + KB - 1) // KB, 1, NKB_MAX).astype(np.int64)
    order = np.argsort(-nkb, kind="stable")
    assignment = np.empty((NCORES, SLOTS), dtype=np.int64)
    for j in range(SLOTS):
        for c in range(NCORES):
            assignment[c, j] = order[j * NCORES + c]
    nkb_slot = tuple(int(nkb[order[j * NCORES]]) for j in range(SLOTS))

    kpos = np.arange(LK)
    in_maps = []
    mmdt = _mm_np_dtype()
    for c in range(NCORES):
        qt_np = np.empty((4, 128, LQ), dtype=mmdt)
        kt_np = np.empty((4, 128, LK), dtype=mmdt)
        va_np = np.empty((SLOTS, NKB_MAX, KB, D + 1), dtype=mmdt)
        mb_np = np.empty((KB, SLOTS * NKB_MAX), dtype=np.float32)
        for j in range(SLOTS):
            b = assignment[c, j]
            p, half = divmod(j, 2)
            qt_np[p, half * 64:(half + 1) * 64, :] = queries[b].T
            kt_np[p, half * 64:(half + 1) * 64, :] = keys[b].T
            va_np[j, :, :, :D] = values[b].reshape(NKB_MAX, KB, D)
            va_np[j, :, :, D] = 1.0
            mb_np[:, j * NKB_MAX:(j + 1) * NKB_MAX] = np.where(
                kpos < vl[b], np.float32(0.0), np.float32(MASK_VALUE)
            ).reshape(NKB_MAX, KB).T
        in_maps.append(
            {
                "qt": np.ascontiguousarray(qt_np),
                "kt": np.ascontiguousarray(kt_np),
                "va": np.ascontiguousarray(va_np),
                "mb": np.ascontiguousarray(mb_np),
            }
        )
    return nkb_slot, in_maps, assignment


def unshard_output(results, assignment):
    out = np.empty((B, LQ, D), dtype=np.float32)
    for c in range(NCORES):
        ot = results[c]["ot"]  # [SLOTS, D, LQ]
        for j in range(SLOTS):
            out[assignment[c, j]] = ot[j].T
    return out


_PROGRAM_CACHE = {}


def _get_program(nkb_slot):
    nc = _PROGRAM_CACHE.get((nkb_slot, MM_DT))
    if nc is None:
        nc = build_program(nkb_slot)
        _PROGRAM_CACHE[(nkb_slot, MM_DT)] = nc
    return nc


def run(inputs, trace=False, **run_kwargs):
    """Shard, run on 8 cores, unshard.  Returns (output, BassKernelResults)."""
    nkb_slot, in_maps, assignment = shard_inputs(**inputs)
    nc = _get_program(nkb_slot)
    res = run_bass_kernel_spmd(
        nc, in_maps, core_ids=list(range(NCORES)), trace=trace, **run_kwargs
    )
    return unshard_output(res.results, assignment), res


def kernel(queries, keys, values, valid_lens):
    out, _ = run(
        {
            "queries": queries,
            "keys": keys,
            "values": values,
            "valid_lens": valid_lens,
        }
    )
    return out

